# revision 30
# baseline (speedup 1.0000x reference)
"""Per-sample dynamic-filter Conv2D (VALID, stride 1) on 8 Trainium2 NeuronCores.

Problem: X [16,128,128,128] (NHWC) conv with per-sample filters
kernel [16,3,3,128,128] (HWIO) -> out [16,126,126,128].

Sharding: pure data parallel — 2 samples per core, no communication.

Device does ONLY the conv matmuls; all layout work lives on the host:
  - Host sends X^T [S, Cin, H*W] bf16 (transpose + downcast in numpy), so
    X^T DMAs into SBUF with contiguous 2KB+ per-partition runs. No
    on-device input transposes.
  - Conv: 9 accumulated bf16 matmuls per output tile (4 output rows x
    126 valid cols via a 3D moving AP [row, ow]) into fp32 PSUM;
    lhsT = filter tap [ci, co], rhs = X^T row windows.
  - fp32 PSUM DMAs straight to DRAM as out^T [S, OH, Cout, OW] (504B
    runs); host transposes to NHWC. No on-device output transposes, no
    PSUM->SBUF copies.
TensorE therefore streams conv matmuls back-to-back at ~1 col/cycle.
"""

import sys

_BASS_PATH = "/opt/trn_rl_repo"
if _BASS_PATH not in sys.path:
    sys.path.insert(0, _BASS_PATH)

import numpy as np

import concourse.mybir as mybir  # noqa: E402
import concourse.tile as tile  # noqa: E402
from concourse import bacc  # noqa: E402

F32 = mybir.dt.float32
BF16 = mybir.dt.bfloat16

# Full-problem constants
B, H, W, CIN, COUT, KH, KW = 16, 128, 128, 128, 128, 3, 3
N_CORES = 8
S = B // N_CORES  # samples per core


def build_conv_nc(S, H, W, C, CO, KH, KW, rows_per_tile=4, rows_per_load=32):
    """Build the per-core Bass program. Returns compiled nc."""
    P = 128
    assert W == P and C == P and CO == P and (H * W) % P == 0
    OH, OW = H - KH + 1, W - KW + 1
    HW = H * W                      # input positions per sample
    RT = rows_per_tile
    NT = (OH + RT - 1) // RT        # output tiles per sample
    RL = rows_per_load
    NG = (H + RL - 1) // RL         # X load groups per sample
    XT_COLS = HW + 2 * P            # pad so rearrange slices stay in bounds

    nc = bacc.Bacc("TRN2", target_bir_lowering=False, debug=False)
    # X^T: [ci, pos] per sample (host-transposed)
    xd = nc.dram_tensor("x", [S, C, HW], BF16, kind="ExternalInput").ap()
    # K^T: [ci, (kh kw co)] per sample (host-transposed, contiguous load)
    kd = nc.dram_tensor("k", [S, C, KH * KW * CO], BF16, kind="ExternalInput").ap()
    # out^T: [co, oh, ow] per sample (host fixes layout after); contiguous
    # (oh, ow) runs per channel make long output-DMA descriptors
    od = nc.dram_tensor("o", [S, CO, OH, OW], BF16, kind="ExternalOutput").ap()

    with tile.TileContext(nc) as tc:
        with (
            tc.tile_pool(name="xt", bufs=2) as xt_pool,
            tc.tile_pool(name="filt", bufs=2) as filt_pool,
            tc.tile_pool(name="ostage", bufs=6) as ostage_pool,
            tc.tile_pool(name="acc", bufs=6, space="PSUM") as acc_pool,
        ):

            def emit_tiles(s, ts, filt, xt):
                """Conv matmuls for a group of tiles, tap-outer: each tap's
                weights are loaded once and stream all tiles in the group
                back-to-back (halves LDWEIGHTS count). Then copy + DMA out
                per tile.

                Tile t covers output rows [t*RT, t*RT+nrows), 126 valid
                columns each (3D moving AP [row, ow] skips the garbage)."""
                accs = {}
                for t in ts:
                    accs[t] = acc_pool.tile([P, RT * OW], F32, tag="acc", name="acc")
                for tap in range(KH * KW):
                    dy, dx = divmod(tap, KW)
                    for t in ts:
                        oh0 = t * RT
                        nrows = min(RT, OH - oh0)
                        n = nrows * OW
                        off = (oh0 + dy) * W + dx
                        rhs = (
                            xt[:, off : off + nrows * P]
                            .rearrange("ci (c w) -> ci c w", c=nrows)[:, :, :OW]
                        )
                        nc.tensor.matmul(
                            accs[t][:, :n],
                            filt[:, tap * CO : (tap + 1) * CO],
                            rhs,
                            start=(tap == 0),
                            stop=(tap == KH * KW - 1),
                        )
                for t in ts:
                    oh0 = t * RT
                    nrows = min(RT, OH - oh0)
                    n = nrows * OW
                    ostage = ostage_pool.tile(
                        [P, RT * OW], BF16, tag="ostage", name="ostage"
                    )
                    # alternate engines so consecutive finishes parallelize
                    if t % 2 == 0:
                        nc.vector.tensor_copy(ostage[:, :n], accs[t][:, :n])
                    else:
                        nc.scalar.copy(ostage[:, :n], accs[t][:, :n])
                    dma_eng = nc.scalar if t % 2 == 0 else nc.sync
                    dma_eng.dma_start(
                        out=od[s, :, oh0 : oh0 + nrows, :],
                        in_=ostage[:, :n].rearrange("co (c w) -> co c w", c=nrows),
                    )

            def emit_group(s, st, r0, nr):
                """DMA X^T rows [r0, r0+nr) of sample s into xt."""
                xt = st["xt"]
                nc.sync.dma_start(
                    out=xt[:, r0 * W : (r0 + nr) * W],
                    in_=xd[s, :, r0 * W : (r0 + nr) * W],
                )

            def load_groups(s):
                """Row-group sizes for sample s's X load. Sample 0 starts
                with a small group so the first tile is ready sooner."""
                sizes = [6, 4, 22] if s == 0 else []
                rem = H - sum(sizes)
                while rem > 0:
                    sizes.append(min(RL, rem))
                    rem -= sizes[-1]
                return sizes

            def sample_actions(s):
                """Loads interleaved with conv tiles at readiness: tile t
                needs X rows < t*RT + nrows + KH - 1."""
                def tile_group(t0):
                    # first two tiles go alone so the first matmuls only
                    # wait on a few X rows; pairs afterwards
                    ts = [t0] if t0 < 2 else [t for t in (t0, t0 + 1) if t < NT]
                    need = max(
                        t * RT + min(RT, OH - t * RT) + KH - 1 for t in ts
                    )
                    return ts, need

                acts = []
                next_t = 0
                rows_loaded = 0
                for nr in load_groups(s):
                    acts.append(("g", rows_loaded, nr))
                    rows_loaded += nr
                    while next_t < NT:
                        ts, need = tile_group(next_t)
                        if need <= rows_loaded:
                            acts.append(("t", ts))
                            next_t += len(ts)
                        else:
                            break
                while next_t < NT:
                    ts, _ = tile_group(next_t)
                    acts.append(("t", ts))
                    next_t += len(ts)
                return acts

            state = {}

            def prelude(s):
                filt = filt_pool.tile(
                    [P, KH * KW * CO], BF16, tag="filt", name="filt"
                )
                # scalar queue: doesn't block the X loads on sync; chunked
                # so early taps land ahead of the matmul stream's consumption
                for lo, hi in ((0, 1), (1, 3), (3, KH * KW)):
                    nc.scalar.dma_start(
                        out=filt[:, lo * CO : hi * CO],
                        in_=kd[s, :, lo * CO : hi * CO],
                    )
                xt = xt_pool.tile([P, XT_COLS], BF16, tag="xt", name="xt")
                state[s] = {"filt": filt, "xt": xt}

            def run_act(s, a):
                st = state[s]
                if a[0] == "g":
                    emit_group(s, st, a[1], a[2])
                else:
                    emit_tiles(s, a[1], st["filt"], st["xt"])

            # Cross-sample software pipeline: sample s+1's prelude and first
            # X load group are emitted before sample s's last TAIL tiles, so
            # the DMA overlaps the tail matmuls.
            TAIL = 3
            all_acts = {s: sample_actions(s) for s in range(S)}
            prelude(0)
            for s in range(S):
                acts = all_acts[s]
                head, tail = (
                    (acts[:-TAIL], acts[-TAIL:]) if s < S - 1 else (acts, [])
                )
                for a in head:
                    run_act(s, a)
                if s < S - 1:
                    prelude(s + 1)
                    nxt = all_acts[s + 1]
                    run_act(s + 1, nxt[0])
                    all_acts[s + 1] = nxt[1:]
                for a in tail:
                    run_act(s, a)

    nc.compile()
    return nc


_NC_CACHE = {}


def _get_nc():
    import os

    rt = int(os.environ.get("CONV_RT", "4"))
    rl = int(os.environ.get("CONV_RL", "32"))
    key = (S, H, W, CIN, COUT, KH, KW, rt, rl)
    if key not in _NC_CACHE:
        _NC_CACHE[key] = build_conv_nc(
            *key[:7], rows_per_tile=rt, rows_per_load=rl
        )
    return _NC_CACHE[key]


def make_in_maps(X, K):
    import ml_dtypes

    # X [B, H, W, Cin] -> X^T [B, Cin, H*W] bf16
    Xt = np.ascontiguousarray(
        X.reshape(B, H * W, CIN).transpose(0, 2, 1)
    ).astype(ml_dtypes.bfloat16)
    # K [B, kh, kw, ci, co] -> [B, ci, kh*kw*co]
    Kb = np.ascontiguousarray(
        K.transpose(0, 3, 1, 2, 4).reshape(B, CIN, KH * KW * COUT)
    ).astype(ml_dtypes.bfloat16)
    return [
        {"x": Xt[i * S : (i + 1) * S], "k": Kb[i * S : (i + 1) * S]}
        for i in range(N_CORES)
    ]


def gather_output(results):
    """Device out^T [S, OH, CO, OW] fp32 per core -> full NHWC [B,OH,OW,CO]."""
    OH, OW = H - KH + 1, W - KW + 1
    out = np.empty((B, OH, OW, COUT), dtype=np.float32)
    for i in range(N_CORES):
        # device layout [S, CO, OH, OW] -> [S, OH, OW, CO]
        out[i * S : (i + 1) * S] = (
            results[i]["o"].astype(np.float32).transpose(0, 2, 3, 1)
        )
    return out


def kernel(**inputs):
    X = np.asarray(inputs["X"], dtype=np.float32)
    K = np.asarray(inputs["kernel"], dtype=np.float32)
    assert X.shape == (B, H, W, CIN), X.shape
    assert K.shape == (B, KH, KW, CIN, COUT), K.shape

    from concourse.bass_utils import run_bass_kernel_spmd

    nc = _get_nc()
    in_maps = make_in_maps(X, K)
    res = run_bass_kernel_spmd(nc, in_maps, list(range(N_CORES)))
    return gather_output(res.results)
